# revision 60
# baseline (speedup 1.0000x reference)
"""Block-sparse attention on 8 Trainium2 NeuronCores (Bass/Tile SPMD kernel).

Sharding: batch*head_groups across the 8 cores. Core c handles batch c//4 and
heads [4*(c%4), 4*(c%4)+4). Projection weights are sliced per core host-side
(pre-transposed + fp16-cast); the [16,16] block mask specializes the compiled
program (only kept blocks are computed). Each core emits a partial output
(its 256-wide d-slice pushed through Wo) in fp16; the host sums the 4
partials per batch in f32 and adds the bias.

Per-core schedule (4 heads = 2 pairs p; all intermediates fp16). The PE
executes in order, so the emission order IS the PE schedule:
  - x^T is DMA'd in 8 contraction chunks; the first k-projection chunk
    accumulates kd-by-kd, pacing with the DMA so the PE starts at ~3us
  - attention runs in 8 units (pair p x 4 q-blocks), software-pipelined at
    flush granularity: scores(f) -> exp(f) on Act -> AV(f) emitted after
    scores(f+1) so the PE never waits on the activation engine
  - all remaining projection work (q0 tail, k1/q1, v) and the first 12
    output-projection tiles are emitted just-in-time inside the flush loops
    as PE gap fillers
  - AV uses the exp'd block as the matmul stationary: av[q,dh] += at^T @ v_j
    (N=64) plus a denominator column via rhs=ones (N=1) into a persistent
    PSUM tile; normalization is reciprocal_approx_fast + one stride-0
    broadcast multiply per head; transposes via PE (identity) into an fp16
    bitcast region of the denominator bank
"""

import os
import time
from contextlib import ExitStack

import numpy as np

import concourse.bass as bass
import concourse.tile as tile
from concourse import bacc, mybir
from concourse.ap import AP as APClass
from concourse.bass_utils import run_bass_kernel_spmd
from concourse.masks import make_identity

F16 = mybir.dt.float16
F32 = mybir.dt.float32
f16 = np.float16

B, S, D, H = 2, 2048, 1024, 16
DH = 64
BLK = 128
NB = 16
NCORES = 8
HPC = H // (NCORES // B)   # 4 heads per core
E = HPC * DH               # 256 projection columns per core
KD = D // 128              # 8 contraction chunks
FILL = 1024                # per-head score fill width (2 PSUM banks)
CB = 4                     # q-blocks per attention unit

_nc_cache: dict = {}
last_run_info: dict = {}


def _runs_of(lst):
    out = []
    for i in lst:
        if out and i == out[-1][-1] + 1:
            out[-1].append(i)
        else:
            out.append([i])
    return out


def _view3(sl, dims, off=0):
    """Raw 3-dim AP view of a 2-dim slice: dims = [[step, n], ...]."""
    return APClass(sl.tensor, sl.offset + off, [list(x) for x in sl.ap[:1]] + dims)


def _plan_flushes(kept, irange):
    """Greedy fill windows: list of (fill_js, colw) per flush for one unit."""
    col_kept = [[i for i in irange if j in kept[i]] for j in range(NB)]
    flushes = []
    fill_js, colw = [], 0
    for j in range(NB):
        ks = col_kept[j]
        while ks:
            room = (FILL - colw) // 128
            if room == 0:
                flushes.append((fill_js, colw))
                fill_js, colw = [], 0
                continue
            take, ks = ks[:room], ks[room:]
            fill_js.append((j, take, colw))
            colw += len(take) * 128
    if fill_js:
        flushes.append((fill_js, colw))
    return flushes


def _emit(tc, aps, kept, dbg_aps=None):
    nc = tc.nc
    xT_ap, wqT_ap, wkT_ap, wvT_ap, woT_ap, outp_ap = aps
    Exp = mybir.ActivationFunctionType.Exp

    first_j = {i: kept[i][0] for i in range(NB)}
    last_j = {i: kept[i][-1] for i in range(NB)}

    with ExitStack() as ctx:
        persist = ctx.enter_context(tc.tile_pool(name="persist", bufs=1))

        # ---- tiles ----------------------------------------------------------
        xT = persist.tile([128, KD * S], F16, name="xT", tag="xT")
        wq = persist.tile([128, KD * E], F16, name="wq", tag="wq")
        wk = persist.tile([128, KD * E], F16, name="wk", tag="wk")
        wv = persist.tile([128, KD * E], F16, name="wv", tag="wv")
        wo = persist.tile([128, 2 * D], F16, name="wo", tag="wo")
        qT = [persist.tile([128, S], F16, name=f"qT{p}", tag=f"qT{p}") for p in range(2)]
        kT = [persist.tile([128, S], F16, name=f"kT{p}", tag=f"kT{p}") for p in range(2)]
        vv = [persist.tile([128, E], F16, name=f"v{m}", tag=f"v{m}") for m in range(S // 128)]
        outT = [persist.tile([128, S], F16, name=f"oT{p}", tag=f"oT{p}") for p in range(2)]
        ones1 = persist.tile([128, 1], F16, name="ones1", tag="ones1")
        nc.vector.memset(ones1[:], 1.0)
        ident = persist.tile([128, 128], F16, name="ident", tag="ident")
        make_identity(nc, ident[:])
        zeros_sb = persist.tile([128, 512], F16, name="zeros_sb", tag="zeros_sb")
        nc.vector.memset(zeros_sb[:], 0.0)
        rc_sb = persist.tile([128, 64], F32, name="rc_sb", tag="rc_sb")

        # ---- loads: wk first, then x^T chunk-by-chunk (paces first k proj) --
        def load_w(dst, src_ap, blocks, width):
            nc.sync.dma_start(
                dst[:].rearrange("p (k s) -> p k s", k=blocks),
                _view3(src_ap[0:128, :], [[128 * width, blocks], [1, width]]),
            )

        def w_piece(dst, src_ap, q):
            hw_ = dst[:, q * 2 * E:(q + 1) * 2 * E]
            nc.sync.dma_start(
                hw_.rearrange("p (k s) -> p k s", k=2),
                _view3(src_ap[0:128, :], [[128 * E, 2], [1, E]],
                       off=q * 2 * 128 * E),
            )

        w_piece(wk, wkT_ap, 0)
        nc.sync.dma_start(xT[:, 0:S], xT_ap[0:128, :])
        w_piece(wq, wqT_ap, 0)
        for kd in range(1, KD):
            nc.sync.dma_start(xT[:, kd * S:(kd + 1) * S],
                              xT_ap[kd * 128:(kd + 1) * 128, :])
            if kd in (1, 3, 5):
                w_piece(wk, wkT_ap, kd // 2 + 1)
                w_piece(wq, wqT_ap, kd // 2 + 1)
        load_w(wv, wvT_ap, KD, E)
        load_w(wo, woT_ap, 2, D)

        proj_pool = ctx.enter_context(tc.tile_pool(name="proj_ps", bufs=2, space="PSUM"))
        fin_sb = ctx.enter_context(tc.tile_pool(name="fin_sb", bufs=8))

        # ---- projection / output emitters (1 PSUM bank each) ----------------
        def qk_chunk(dst, w, p, scw):
            ps = proj_pool.tile([128, 512], F32, name="projqk", tag="proj")
            for kd in range(KD):
                nc.tensor.matmul(
                    ps[:],
                    w[:, kd * E + p * 128: kd * E + (p + 1) * 128],
                    xT[:, kd * S + scw * 512: kd * S + (scw + 1) * 512],
                    start=(kd == 0),
                    stop=(kd == KD - 1),
                )
            nc.vector.tensor_copy(dst[:, scw * 512:(scw + 1) * 512], ps[:])

        def v_proj(m):
            ps = proj_pool.tile([128, 512], F32, name="projv", tag="proj")
            for kd in range(KD):
                nc.tensor.matmul(
                    ps[:, 0:E],
                    xT[:, kd * S + m * 128: kd * S + (m + 1) * 128],
                    wv[:, kd * E:(kd + 1) * E],
                    start=(kd == 0),
                    stop=(kd == KD - 1),
                )
            nc.vector.tensor_copy(vv[m][:], ps[:, 0:E])

        def fin(m, pool=None, act=False):
            pool = pool or proj_pool
            pss = [pool.tile([128, 512], F32, name="finps", tag="proj")
                   for _ in range(2)]
            for p in range(2):
                for n in range(2):
                    nc.tensor.matmul(
                        pss[n][:],
                        outT[p][:, m * 128:(m + 1) * 128],
                        wo[:, p * D + n * 512: p * D + (n + 1) * 512],
                        start=(p == 0),
                        stop=(p == 1),
                    )
            st = fin_sb.tile([128, 1024], F16, name="finst", tag="finsb")
            nc.vector.tensor_copy(st[:, 0:512], pss[0][:])
            if act:
                nc.scalar.copy(st[:, 512:1024], pss[1][:])
            else:
                nc.vector.tensor_copy(st[:, 512:1024], pss[1][:])
            nc.sync.dma_start(outp_ap[m * 128:(m + 1) * 128, :], st[:])

        # q0 + k0 kd-outer in an 8-bank scratch pool: each x^T chunk DMA
        # unlocks 8 matmuls, so the PE stays busy while the load streams in
        with tc.tile_pool(name="head_ps", bufs=1, space="PSUM") as head_pool:
            hps = [head_pool.tile([128, 512], F32, name=f"hps{t}", tag=f"hps{t}")
                   for t in range(8)]
            for kd in range(KD):
                for t in range(8):
                    w, dst_scw = (wk, t) if t < 4 else (wq, t - 4)
                    nc.tensor.matmul(
                        hps[t][:],
                        w[:, kd * E: kd * E + 128],
                        xT[:, kd * S + dst_scw * 512: kd * S + (dst_scw + 1) * 512],
                        start=(kd == 0),
                        stop=(kd == KD - 1),
                    )
            for t in range(8):
                dst = kT[0] if t < 4 else qT[0]
                scw = t if t < 4 else t - 4
                if t % 2 == 0:
                    nc.vector.tensor_copy(dst[:, scw * 512:(scw + 1) * 512], hps[t][:])
                else:
                    nc.scalar.copy(dst[:, scw * 512:(scw + 1) * 512], hps[t][:])

        # ---- attention -------------------------------------------------------
        with ExitStack() as actx:
            sc_pool = actx.enter_context(tc.tile_pool(name="sc_ps", bufs=1, space="PSUM"))
            av_pool = actx.enter_context(tc.tile_pool(name="av_ps", bufs=1, space="PSUM"))
            dt_pool = actx.enter_context(tc.tile_pool(name="dt_ps", bufs=1, space="PSUM"))
            at_pool = actx.enter_context(tc.tile_pool(name="at_sb", bufs=12))
            on_pool = actx.enter_context(tc.tile_pool(name="on_sb", bufs=5))

            # shared bank: denominator columns (f32) + transpose region (f16)
            dt = dt_pool.tile([128, 512], F32, name="dt", tag="dt")
            den = dt[:, 0:64]
            tp_f16 = dt[:, 128:384].bitcast(F16)  # [128, 512] f16
            nc.tensor.matmul(den, ident[:], zeros_sb[:, 0:64],
                             start=True, stop=False, skip_group_check=True)
            v_emitted = set()

            def emit_scores(p, scs, fill_js):
                for j, ks, off in fill_js:
                    for a in range(2):
                        rows = slice(64 * a, 64 * a + 64)
                        for run in _runs_of(ks):
                            idx0 = ks.index(run[0])
                            col = off + idx0 * 128
                            width = len(run) * 128
                            qcol = run[0] * 128
                            done = 0
                            while done < width:
                                seg = min(width - done, 512 - ((col + done) % 512))
                                nc.tensor.matmul(
                                    scs[a][:, col + done: col + done + seg],
                                    kT[p][rows, j * 128:(j + 1) * 128],
                                    qT[p][rows, qcol + done: qcol + done + seg],
                                )
                                done += seg

            def emit_av(dbase, p, irange, av, ats, fill_js):
                nb = len(irange)
                for j, ks, off in fill_js:
                    for a in range(2):
                        h = 2 * p + a
                        for idx, i in enumerate(ks):
                            il = i - irange[0]
                            blk = ats[a][:, off + idx * 128: off + (idx + 1) * 128]
                            nc.tensor.matmul(
                                av[:, a * 256 + il * 64: a * 256 + (il + 1) * 64],
                                blk,
                                vv[j][:, h * 64:(h + 1) * 64],
                                start=False,
                                stop=(j == last_j[i]),
                                skip_group_check=True,
                            )
                            c = dbase + a * nb + il
                            nc.tensor.matmul(
                                den[:, c: c + 1],
                                blk,
                                ones1[:],
                                start=False,
                                stop=(j == last_j[i]),
                                skip_group_check=True,
                            )

            def close_unit(dbase, p, irange, av, outN):
                """Normalization for a finished unit + deferred transpose."""
                nb = len(irange)
                nc.vector.reciprocal_approx_fast(rc_sb[:, dbase:dbase + 2 * nb],
                                                 den[:, dbase:dbase + 2 * nb])
                for a in range(2):
                    rcs = rc_sb[:, dbase + a * nb: dbase + (a + 1) * nb]
                    rc_b = APClass(rcs.tensor, rcs.offset,
                                   [list(rcs.ap[0]), list(rcs.ap[1]), [0, 64]])
                    nc.vector.tensor_mul(
                        _view3(outN[:, a * 64:], [[128, nb], [1, 64]]),
                        _view3(av[:, a * 256:], [[64, nb], [1, 64]]),
                        rc_b,
                    )

                def finish():
                    for il in range(nb):
                        nc.tensor.matmul(
                            tp_f16[:, il * 128:(il + 1) * 128],
                            outN[:, il * 128:(il + 1) * 128],
                            ident[:],
                            is_transpose=True,
                            skip_group_check=True,
                        )
                    nc.vector.tensor_copy(
                        outT[p][:, irange[0] * 128:(irange[-1] + 1) * 128],
                        tp_f16[:, 0:nb * 128])
                    done.add(dbase)

                return finish

            # Unit list with PE gap fillers: k1/q1 ahead of the p=1 units,
            # then output-projection tiles as their outT cols land. The last
            # unit is split in two so its finalization chain (and the m=12,13
            # output tiles) overlap the final flushes instead of trailing.
            units = [
                (0, 0, [0, 1, 2, 3], []),
                (8, 0, [4, 5, 6, 7], [(None, lambda: qk_chunk(kT[1], wk, 1, 0))]),
                (16, 0, [8, 9, 10, 11], [(None, lambda: qk_chunk(kT[1], wk, 1, 1))]),
                (24, 0, [12, 13, 14, 15], [(None, lambda: qk_chunk(qT[1], wq, 1, 0))]),
                (32, 1, [0, 1, 2, 3], [(None, lambda: qk_chunk(kT[1], wk, 1, 2)),
                                       (None, lambda: qk_chunk(kT[1], wk, 1, 3)),
                                       (None, lambda: qk_chunk(qT[1], wq, 1, 1)),
                                       (None, lambda: qk_chunk(qT[1], wq, 1, 2)),
                                       (None, lambda: qk_chunk(qT[1], wq, 1, 3))]),
                (40, 1, [4, 5, 6, 7], [(32, lambda m=m: fin(m)) for m in range(0, 4)]),
                (48, 1, [8, 9, 10, 11], [(40, lambda m=m: fin(m)) for m in range(4, 8)]),
                (56, 1, [12, 13], [(48, lambda m=m: fin(m, act=True)) for m in range(8, 12)]),
                (60, 1, [14, 15], [(56, lambda m=m: fin(m, act=True)) for m in (12, 13)]),
            ]
            items = []
            for dbase, p, irange, ex in units:
                fl = _plan_flushes(kept, irange)
                nfl = len(fl)
                for fi, (fill_js, colw) in enumerate(fl):
                    # extras attach at the unit's 2nd item: by then the
                    # previous unit's deferred finish (which output-projection
                    # fillers depend on) has been popped ahead of them
                    items.append((dbase, p, irange, fill_js, colw, fi == 0,
                                  fi == nfl - 1,
                                  ex if fi == min(1, nfl - 1) else None))

            # Global software pipeline across ALL units: AV(f) is emitted
            # after scores(f+1) even across unit boundaries, so the PE never
            # drains at a unit seam waiting for the last exp.
            ustate = {}
            extras = []
            deferred = []   # (appended_at_item_idx, finish closure)
            pending = []    # flushes awaiting AV emission (depth-3 pipeline)
            done = set()    # dbase of units whose finish has been emitted
            prev = None

            def handle_prev(idx):
                dbase, p, irange, ats, fill_js, first, last = prev
                if first:
                    av = av_pool.tile([128, 512], F32, name="av", tag="av")
                    nc.tensor.matmul(av[:], ident[:], zeros_sb[:],
                                     start=True, stop=False, skip_group_check=True)
                    outN = on_pool.tile([128, 512], F16, name="outN", tag="outN")
                    ustate[dbase] = (av, outN)
                av, outN = ustate[dbase]
                emit_av(dbase, p, irange, av, ats, fill_js)
                if last:
                    deferred.append((idx, close_unit(dbase, p, irange, av, outN)))

            for idx, (dbase, p, irange, fill_js, colw, first, last, ex) in enumerate(items):
                if ex:
                    extras.extend(ex)
                popped_def = False
                if deferred and deferred[0][0] < idx:
                    deferred.pop(0)[1]()
                    popped_def = True
                if p == 0:
                    for j, _, _ in fill_js:
                        if j not in v_emitted:
                            v_emitted.add(j)
                            v_proj(j)
                if (extras and not deferred
                        and (extras[0][0] is None or extras[0][0] in done)):
                    extras.pop(0)[1]()
                scs = [sc_pool.tile([128, FILL], F32, name=f"sc{a}", tag=f"sc{a}")
                       for a in range(2)]
                emit_scores(p, scs, fill_js)
                ats = []
                for a in range(2):
                    at = at_pool.tile([128, FILL], F16, name="at", tag="at")
                    nc.scalar.activation(at[:, 0:colw], scs[a][:, 0:colw], Exp)
                    ats.append(at)
                if len(pending) == 3:
                    prev = pending.pop(0)
                    handle_prev(idx)
                pending.append((dbase, p, irange, ats, fill_js, first, last))
            while pending:
                prev = pending.pop(0)
                handle_prev(len(items))
            while deferred:
                deferred.pop(0)[1]()
            while extras:
                extras.pop(0)[1]()
            if dbg_aps is not None:
                den_sb = persist.tile([128, 64], F32, name="den_sb", tag="den_sb")
                nc.vector.tensor_copy(den_sb[:], den)
                nc.sync.dma_start(dbg_aps["d_den"][:, :], den_sb[:])

        # ---- remaining output projection (deeper PSUM buffering in the tail)
        with tc.tile_pool(name="tail_ps", bufs=6, space="PSUM") as tail_pool:
            for m in range(14, S // 128):
                fin(m, tail_pool, act=True)

        if dbg_aps is not None:
            for n, t in [("d_qT0", qT[0]), ("d_kT0", kT[0]),
                         ("d_qT1", qT[1]), ("d_kT1", kT[1]),
                         ("d_rc", rc_sb), ("d_oT0", outT[0]), ("d_oT1", outT[1])]:
                nc.sync.dma_start(dbg_aps[n][:, :], t[:])
            for m in range(16):
                nc.sync.dma_start(dbg_aps["d_v"][:, m * E:(m + 1) * E], vv[m][:])


def _get_nc(kept):
    key = kept
    if key in _nc_cache:
        return _nc_cache[key]
    nc = bacc.Bacc("TRN2", target_bir_lowering=False, debug=False, num_devices=NCORES)
    xT_ap = nc.dram_tensor("xT", [D, S], F16, kind="ExternalInput").ap()
    wqT_ap = nc.dram_tensor("wqT", [D, E], F16, kind="ExternalInput").ap()
    wkT_ap = nc.dram_tensor("wkT", [D, E], F16, kind="ExternalInput").ap()
    wvT_ap = nc.dram_tensor("wvT", [D, E], F16, kind="ExternalInput").ap()
    woT_ap = nc.dram_tensor("woT", [E, D], F16, kind="ExternalInput").ap()
    outp_ap = nc.dram_tensor("outp", [S, D], F16, kind="ExternalOutput").ap()
    dbg_aps = None
    if os.environ.get("KDBG") == "1":
        dbg_aps = {
            n: nc.dram_tensor(n, shp, dt, kind="ExternalOutput").ap()
            for n, shp, dt in [
                ("d_qT0", [128, S], F16), ("d_kT0", [128, S], F16),
                ("d_qT1", [128, S], F16), ("d_kT1", [128, S], F16),
                ("d_v", [128, 16 * E], F16), ("d_den", [128, 64], F32),
                ("d_rc", [128, 64], F32),
                ("d_oT0", [128, S], F16), ("d_oT1", [128, S], F16),
            ]
        }
    with tile.TileContext(nc) as tc:
        _emit(tc, (xT_ap, wqT_ap, wkT_ap, wvT_ap, woT_ap, outp_ap), kept,
              dbg_aps=dbg_aps)
    nc.compile()
    _nc_cache[key] = nc
    return nc


def kernel(x, Wq, Wk, Wv, Wo, bo, block_mask):
    x = np.asarray(x, dtype=np.float32)
    Wq = np.asarray(Wq, dtype=np.float32)
    Wk = np.asarray(Wk, dtype=np.float32)
    Wv = np.asarray(Wv, dtype=np.float32)
    Wo = np.asarray(Wo, dtype=np.float32)
    bo = np.asarray(bo, dtype=np.float32)
    mask = np.asarray(block_mask).astype(bool)

    kept = tuple(tuple(int(j) for j in np.nonzero(mask[i])[0]) for i in range(NB))
    assert all(len(js) > 0 for js in kept), "a query block row has no kept blocks"

    t0 = time.monotonic()
    nc = _get_nc(kept)
    t_compile = time.monotonic() - t0

    xT_b = [np.ascontiguousarray(x[b].T).astype(f16) for b in range(B)]
    in_maps = []
    for c in range(NCORES):
        b = c // (NCORES // B)
        hs = c % (NCORES // B)
        sl = slice(hs * E, (hs + 1) * E)
        in_maps.append({
            "xT": xT_b[b],
            "wqT": np.ascontiguousarray((Wq[sl, :] / np.sqrt(np.float32(DH))).T).astype(f16),
            "wkT": np.ascontiguousarray(Wk[sl, :].T).astype(f16),
            "wvT": np.ascontiguousarray(Wv[sl, :].T).astype(f16),
            "woT": np.ascontiguousarray(Wo[:, sl].T).astype(f16),
        })

    t0 = time.monotonic()
    res = run_bass_kernel_spmd(nc, in_maps, list(range(NCORES)))
    t_run = time.monotonic() - t0

    out = np.zeros((B, S, D), np.float32)
    for c in range(NCORES):
        out[c // (NCORES // B)] += res.results[c]["outp"].astype(np.float32)
    out += bo[None, None, :]

    last_run_info.update(compile_s=t_compile, run_s=t_run, nc=nc)
    return out
